# revision 28
# baseline (speedup 1.0000x reference)
"""BasicRNN+LN Trainium2 kernel.

Math (per step t):
    h_norm = LayerNorm(h)            (no affine)
    I      = h_norm @ Wh2h.T + x_t @ Wi2h.T
    fh     = sigmoid(I)
    h_new  = 0.9*h + 0.1*fh
    y_t    = h_new @ Wh2y.T

Key trick: fold the LN mean-subtraction into the recurrent weights:
    W'[j,k] = Wh2h[j,k] - rowsum(Wh2h)[j]/nh
    h_norm @ Wh2h.T == rstd * (h @ W'.T)
so the PE consumes raw h (transposed) and only a per-partition rstd
scalar is needed on the DVE.

Sharding: batch 256 -> 32 per core across 8 cores; weights replicated;
the T=2048 scan stays local per core.

Layouts per core (B=32):
  h state lives as the rolling slice of the hseq chunk buffer [32, TC*256]
  (partition=batch) plus a transposed copy hseqT [128, TC*64]
  (partition=k, col = t*64 + kchunk*32 + b) used as the matmul stationary.
"""

import numpy as np

FP32 = None  # set lazily (mybir import is heavy)

B, T, NX, NH, NY = 256, 2048, 64, 256, 64
NCORES = 8
BL = B // NCORES          # 32 batch rows per core
TC = 64                   # timesteps per chunk
NCHUNK = T // TC
ALPHA = 0.1
LN_EPS = 1e-5

TRACE = False             # test.py sets this for profiled runs
LAST_RESULT = {}          # exec_time_ns etc for test.py


def build_bass():
    import concourse.bass as bass
    import concourse.bacc as bacc
    import concourse.mybir as mybir
    from concourse.tile import TileContext

    fp = mybir.dt.float32
    AL = mybir.AluOpType
    AF = mybir.ActivationFunctionType

    nc = bacc.Bacc()

    x = nc.dram_tensor("x", [BL, T, NX], fp, kind="ExternalInput")
    h0 = nc.dram_tensor("h0", [BL, NH], fp, kind="ExternalInput")
    wpT_d = nc.dram_tensor("wpT", [NH, NH], fp, kind="ExternalInput")
    wiT_d = nc.dram_tensor("wiT", [NX, NH], fp, kind="ExternalInput")
    wyT_d = nc.dram_tensor("wyT", [NH, NY], fp, kind="ExternalInput")
    ident_d = nc.dram_tensor("ident", [BL, BL], fp, kind="ExternalInput")
    y_d = nc.dram_tensor("y", [BL, T, NY], fp, kind="ExternalOutput")
    hseq_d = nc.dram_tensor("hseq", [BL, T, NH], fp, kind="ExternalOutput")

    with TileContext(nc) as tc:
        with (
            tc.tile_pool(name="const", bufs=1) as cp,
            tc.tile_pool(name="state", bufs=1) as sp,
            tc.tile_pool(name="xin", bufs=2) as xp,
            tc.tile_pool(name="work", bufs=3) as wk,
            tc.tile_pool(name="pmm", bufs=2, space="PSUM") as pmm,
            tc.tile_pool(name="pt", bufs=1, space="PSUM") as pt,
            tc.tile_pool(name="py", bufs=2, space="PSUM") as py,
        ):
            # ---- constants ----
            wp0 = cp.tile([128, NH], fp, tag="wp0")
            wp1 = cp.tile([128, NH], fp, tag="wp1")
            wi = cp.tile([NX, NH], fp, tag="wi")
            wy0 = cp.tile([128, NY], fp, tag="wy0")
            wy1 = cp.tile([128, NY], fp, tag="wy1")
            idt = cp.tile([BL, BL], fp, tag="idt")
            epsc = cp.tile([BL, 1], fp, tag="epsc")
            nc.vector.memset(epsc[:], LN_EPS)
            nc.sync.dma_start(wp0[:], wpT_d[0:128, :])
            nc.sync.dma_start(wp1[:], wpT_d[128:256, :])
            nc.sync.dma_start(wi[:], wiT_d[:, :])
            nc.sync.dma_start(wy0[:], wyT_d[0:128, :])
            nc.sync.dma_start(wy1[:], wyT_d[128:256, :])
            nc.sync.dma_start(idt[:], ident_d[:, :])

            # ---- persistent buffers ----
            hseq_sb = sp.tile([BL, TC * NH], fp, tag="hseq")      # 64KB/part
            hseqT_sb = sp.tile([128, TC * 2 * BL], fp, tag="hseqT")  # 16KB/part
            xT_sb = sp.tile([NX, TC * BL], fp, tag="xT")          # 8KB/part
            y_sb = sp.tile([128, (TC // 4) * NY], fp, tag="ysb")  # 4KB/part
            h9 = [sp.tile([BL, NH], fp, tag=f"h9{i}", name=f"h9{i}")
                  for i in range(2)]
            rstd = [sp.tile([BL, 1], fp, tag=f"rstd{i}", name=f"rstd{i}")
                    for i in range(2)]

            # ---- bootstrap: place h0 at slice TC-1, fill T-state/stats ----
            last = TC - 1
            nc.sync.dma_start(hseq_sb[:, last * NH:(last + 1) * NH], h0[:, :])
            hT_ps0 = pt.tile([128, 2 * BL], fp, tag="hT")
            nc.tensor.transpose(
                hT_ps0[:, 0:BL], hseq_sb[:, last * NH:last * NH + 128], idt[:, :])
            nc.tensor.transpose(
                hT_ps0[:, BL:2 * BL],
                hseq_sb[:, last * NH + 128:(last + 1) * NH], idt[:, :])
            hsv0 = hseqT_sb.rearrange("p (g c x) -> p g c x", c=2, x=4 * BL)
            nc.scalar.copy(
                hsv0[:, last // 4, :, (last % 4) * BL:(last % 4 + 1) * BL],
                hT_ps0.rearrange("p (c b) -> p c b", c=2))
            hprev0 = hseq_sb[:, last * NH:(last + 1) * NH]
            nc.gpsimd.tensor_scalar_mul(h9[1][:], hprev0, 1.0 - ALPHA)
            st6b = wk.tile([BL, 6], fp, tag="st6")
            st2b = wk.tile([BL, 2], fp, tag="st2")
            sveb = wk.tile([BL, 1], fp, tag="sve")
            nc.vector.bn_stats(st6b[:], hprev0)
            nc.vector.bn_aggr(st2b[:], st6b[:])
            nc.scalar.activation(sveb[:], st2b[:, 1:2], AF.Sqrt, bias=epsc[:])
            nc.vector.reciprocal(rstd[1][:], sveb[:])

            ET = mybir.EngineType
            with tc.For_i(0, T, TC,
                          hint_engines=(ET.PE, ET.Activation, ET.DVE,
                                        ET.Pool, ET.SP)) as iv:
                # x chunk in
                xbuf = xp.tile([BL, TC * NX], fp, tag="xbuf")
                nc.sync.dma_start(xbuf[:], x[:, bass.ds(iv, TC), :])

                # x transpose prepass: [32,64] per t -> xT [64, t*32+b]
                for g in range(TC // 2):
                    xt_ps = pmm.tile([NX, 2 * BL], fp, tag="xt", bufs=1)
                    for u in range(2):
                        t = 2 * g + u
                        nc.tensor.transpose(
                            xt_ps[:, u * BL:(u + 1) * BL],
                            xbuf[:, t * NX:(t + 1) * NX], idt[:, :])
                    nc.scalar.copy(
                        xT_sb[:, 2 * g * BL:(2 * g + 2) * BL], xt_ps[:])

                # X2 = x_t @ Wi2h.T, pairs of steps share one PSUM bank,
                # then one ACT copy to SBUF (STT reads only 1 PSUM input,
                # and partition bases of its SB inputs must match)
                x2_tiles = []
                for q in range(TC // 2):
                    x2_ps = pmm.tile([BL, 2 * NH], fp, tag="x2")
                    for u in range(2):
                        nc.tensor.matmul(
                            x2_ps[:, u * NH:(u + 1) * NH],
                            xT_sb[:, (2 * q + u) * BL:(2 * q + u + 1) * BL],
                            wi[:], start=True, stop=True)
                    x2c = wk.tile([BL, 2 * NH], fp, tag="x2sb", bufs=4,
                                  name=f"x2c{q}")
                    nc.scalar.copy(x2c[:], x2_ps[:])
                    x2_tiles.append(x2c)

                y_ps = None
                for t in range(TC):
                    pv = (t + TC - 1) % TC
                    cur = t % 2
                    prv = (t + 1) % 2
                    h_prev = hseq_sb[:, pv * NH:(pv + 1) * NH]
                    # hseqT col layout: (t//4)*256 + c*128 + (t%4)*32 + b
                    hsv = hseqT_sb.rearrange("p (g c x) -> p g c x",
                                             c=2, x=4 * BL)
                    gp, tp = pv // 4, pv % 4

                    # G = h_prev @ W'.T  (2 K-chunks accumulate)
                    g_ps = pmm.tile([BL, NH], fp, tag="g")
                    nc.tensor.matmul(
                        g_ps[:], hsv[:, gp, 0, tp * BL:(tp + 1) * BL],
                        wp0[:], start=True, stop=False)
                    nc.tensor.matmul(
                        g_ps[:], hsv[:, gp, 1, tp * BL:(tp + 1) * BL],
                        wp1[:], start=False, stop=True)
                    # z = G*rstd + X2
                    z = wk.tile([BL, NH], fp, tag="z")
                    x2c = x2_tiles[t // 2]
                    nc.vector.scalar_tensor_tensor(
                        z[:], g_ps[:], rstd[prv][:],
                        x2c[:, (t % 2) * NH:(t % 2 + 1) * NH],
                        op0=AL.mult, op1=AL.add)
                    # fh = sigmoid(z)
                    fh = wk.tile([BL, NH], fp, tag="fh")
                    nc.scalar.activation(fh[:], z[:], AF.Sigmoid)
                    # h_new = 0.1*fh + 0.9*h_prev  -> hseq slice t
                    h_new = hseq_sb[:, t * NH:(t + 1) * NH]
                    nc.vector.scalar_tensor_tensor(
                        h_new, fh[:], ALPHA, h9[prv][:],
                        op0=AL.mult, op1=AL.add)
                    # 0.9*h_new for next step (gpsimd)
                    nc.gpsimd.tensor_scalar_mul(h9[cur][:], h_new, 1.0 - ALPHA)
                    # rstd of h_new
                    st6 = wk.tile([BL, 6], fp, tag="st6")
                    nc.vector.bn_stats(st6[:], h_new)
                    st2 = wk.tile([BL, 2], fp, tag="st2")
                    nc.vector.bn_aggr(st2[:], st6[:])
                    sve = wk.tile([BL, 1], fp, tag="sve")
                    nc.scalar.activation(sve[:], st2[:, 1:2], AF.Sqrt,
                                         bias=epsc[:])
                    nc.vector.reciprocal(rstd[cur][:], sve[:])
                    # transpose h_new into T-state
                    hT_ps = pt.tile([128, 2 * BL], fp, tag="hT")
                    nc.tensor.transpose(
                        hT_ps[:, 0:BL], hseq_sb[:, t * NH:t * NH + 128],
                        idt[:, :])
                    nc.tensor.transpose(
                        hT_ps[:, BL:2 * BL],
                        hseq_sb[:, t * NH + 128:(t + 1) * NH], idt[:, :])
                    nc.scalar.copy(
                        hsv[:, t // 4, :, (t % 4) * BL:(t % 4 + 1) * BL],
                        hT_ps.rearrange("p (c b) -> p c b", c=2))
                    # y = h_new @ Wh2y.T, M-packed 4 steps: rows (tt,b)=128,
                    # de-interleave happens in the output DMA access pattern
                    if t % 4 == 3:
                        q = t // 4
                        y_ps = py.tile([128, NY], fp, tag="y")
                        for c in range(2):
                            nc.tensor.matmul(
                                y_ps[:], hsv[:, t // 4, c, :],
                                [wy0, wy1][c][:],
                                start=(c == 0), stop=(c == 1))
                        nc.scalar.copy(y_sb[:, q * NY:(q + 1) * NY], y_ps[:])
                    # hseq/y halves out
                    if t == TC // 2 - 1:
                        nc.sync.dma_start(
                            hseq_d[:, bass.ds(iv, TC // 2), :],
                            hseq_sb[:, 0:(TC // 2) * NH])
                    elif t == TC - 1:
                        nc.sync.dma_start(
                            hseq_d[:, bass.ds(iv + TC // 2, TC // 2), :],
                            hseq_sb[:, (TC // 2) * NH:])
                        yv = y_d[:, bass.ds(iv, TC), :].rearrange(
                            "b (q tt) j -> tt b q j", tt=4)
                        for tt in range(4):
                            nc.sync.dma_start(
                                yv[tt], y_sb[tt * BL:(tt + 1) * BL, :])
    nc.finalize()
    return nc


_NC_CACHE = None


def kernel(x, h, Wi2h, Wh2h, Wh2y):
    global _NC_CACHE, LAST_RESULT
    from concourse import bass_utils

    x = np.ascontiguousarray(np.asarray(x, dtype=np.float32))
    h = np.ascontiguousarray(np.asarray(h, dtype=np.float32))
    Wi2h = np.asarray(Wi2h, dtype=np.float32)
    Wh2h = np.asarray(Wh2h, dtype=np.float32)
    Wh2y = np.asarray(Wh2y, dtype=np.float32)

    rs = Wh2h.sum(axis=1, keepdims=True)
    wpT = np.ascontiguousarray((Wh2h - rs / NH).T)
    wiT = np.ascontiguousarray(Wi2h.T)
    wyT = np.ascontiguousarray(Wh2y.T)
    ident = np.eye(BL, dtype=np.float32)

    if _NC_CACHE is None:
        _NC_CACHE = build_bass()
    nc = _NC_CACHE

    in_maps = []
    for c in range(NCORES):
        in_maps.append({
            "x": np.ascontiguousarray(x[c * BL:(c + 1) * BL]),
            "h0": np.ascontiguousarray(h[c * BL:(c + 1) * BL]),
            "wpT": wpT, "wiT": wiT, "wyT": wyT, "ident": ident,
        })

    res = bass_utils.run_bass_kernel_spmd(
        nc, in_maps, core_ids=list(range(NCORES)), trace=TRACE)
    LAST_RESULT = {"exec_time_ns": res.exec_time_ns,
                   "trace": res.instructions_and_trace}
    y = np.concatenate([r["y"] for r in res.results], axis=0)
    hseq = np.concatenate([r["hseq"] for r in res.results], axis=0)
    return y, hseq


# revision 29
# speedup vs baseline: 1.6150x; 1.6150x over previous
"""BasicRNN+LN Trainium2 kernel.

Math (per step t):
    h_norm = LayerNorm(h)            (no affine)
    I      = h_norm @ Wh2h.T + x_t @ Wi2h.T
    fh     = sigmoid(I)
    h_new  = 0.9*h + 0.1*fh
    y_t    = h_new @ Wh2y.T

Key trick: fold the LN mean-subtraction into the recurrent weights:
    W'[j,k] = Wh2h[j,k] - rowsum(Wh2h)[j]/nh
    h_norm @ Wh2h.T == rstd * (h @ W'.T)
so the PE consumes raw h (transposed) and only a per-partition rstd
scalar is needed on the DVE.

Sharding: batch 256 -> 32 per core across 8 cores; weights replicated;
the T=2048 scan stays local per core.

Layouts per core (B=32):
  h state lives as the rolling slice of the hseq chunk buffer [32, TC*256]
  (partition=batch) plus a transposed copy hseqT [128, TC*64]
  (partition=k, col = t*64 + kchunk*32 + b) used as the matmul stationary.
"""

import numpy as np

FP32 = None  # set lazily (mybir import is heavy)

B, T, NX, NH, NY = 256, 2048, 64, 256, 64
NCORES = 8
BL = B // NCORES          # 32 batch rows per core
TC = 64                   # timesteps per chunk
NCHUNK = T // TC
ALPHA = 0.1
LN_EPS = 1e-5

TRACE = False             # test.py sets this for profiled runs
LAST_RESULT = {}          # exec_time_ns etc for test.py


def build_bass():
    import concourse.bass as bass
    import concourse.bacc as bacc
    import concourse.mybir as mybir
    from concourse.tile import TileContext

    fp = mybir.dt.float32
    AL = mybir.AluOpType
    AF = mybir.ActivationFunctionType

    nc = bacc.Bacc()

    x = nc.dram_tensor("x", [BL, T, NX], fp, kind="ExternalInput")
    h0 = nc.dram_tensor("h0", [BL, NH], fp, kind="ExternalInput")
    wpT_d = nc.dram_tensor("wpT", [NH, NH], fp, kind="ExternalInput")
    wiT_d = nc.dram_tensor("wiT", [NX, NH], fp, kind="ExternalInput")
    wyT_d = nc.dram_tensor("wyT", [NH, NY], fp, kind="ExternalInput")
    ident_d = nc.dram_tensor("ident", [BL, BL], fp, kind="ExternalInput")
    y_d = nc.dram_tensor("y", [BL, T, NY], fp, kind="ExternalOutput")
    hseq_d = nc.dram_tensor("hseq", [BL, T, NH], fp, kind="ExternalOutput")

    with TileContext(nc) as tc:
        with (
            tc.tile_pool(name="const", bufs=1) as cp,
            tc.tile_pool(name="state", bufs=1) as sp,
            tc.tile_pool(name="xin", bufs=2) as xp,
            tc.tile_pool(name="work", bufs=3) as wk,
            tc.tile_pool(name="pmm", bufs=2, space="PSUM") as pmm,
            tc.tile_pool(name="pt", bufs=1, space="PSUM") as pt,
            tc.tile_pool(name="py", bufs=2, space="PSUM") as py,
        ):
            # ---- constants ----
            wp0 = cp.tile([128, NH], fp, tag="wp0")
            wp1 = cp.tile([128, NH], fp, tag="wp1")
            wi = cp.tile([NX, NH], fp, tag="wi")
            wy0 = cp.tile([128, NY], fp, tag="wy0")
            wy1 = cp.tile([128, NY], fp, tag="wy1")
            idt = cp.tile([BL, BL], fp, tag="idt")
            epsc = cp.tile([BL, 1], fp, tag="epsc")
            nc.vector.memset(epsc[:], LN_EPS)
            nc.sync.dma_start(wp0[:], wpT_d[0:128, :])
            nc.sync.dma_start(wp1[:], wpT_d[128:256, :])
            nc.sync.dma_start(wi[:], wiT_d[:, :])
            nc.sync.dma_start(wy0[:], wyT_d[0:128, :])
            nc.sync.dma_start(wy1[:], wyT_d[128:256, :])
            nc.sync.dma_start(idt[:], ident_d[:, :])

            # ---- persistent buffers ----
            hseq_sb = sp.tile([BL, TC * NH], fp, tag="hseq")      # 64KB/part
            hseqT_sb = sp.tile([128, TC * 2 * BL], fp, tag="hseqT")  # 16KB/part
            xT_sb = sp.tile([NX, TC * BL], fp, tag="xT")          # 8KB/part
            y_sb = sp.tile([128, (TC // 4) * NY], fp, tag="ysb")  # 4KB/part
            h9 = [sp.tile([BL, NH], fp, tag=f"h9{i}", name=f"h9{i}")
                  for i in range(2)]
            rstd = [sp.tile([BL, 1], fp, tag=f"rstd{i}", name=f"rstd{i}")
                    for i in range(2)]

            # ---- bootstrap: place h0 at slice TC-1, fill T-state/stats ----
            last = TC - 1
            nc.sync.dma_start(hseq_sb[:, last * NH:(last + 1) * NH], h0[:, :])
            hT_ps0 = pt.tile([128, 2 * BL], fp, tag="hT")
            nc.tensor.transpose(
                hT_ps0[:, 0:BL], hseq_sb[:, last * NH:last * NH + 128], idt[:, :])
            nc.tensor.transpose(
                hT_ps0[:, BL:2 * BL],
                hseq_sb[:, last * NH + 128:(last + 1) * NH], idt[:, :])
            hsv0 = hseqT_sb.rearrange("p (g c x) -> p g c x", c=2, x=4 * BL)
            nc.scalar.copy(
                hsv0[:, last // 4, :, (last % 4) * BL:(last % 4 + 1) * BL],
                hT_ps0.rearrange("p (c b) -> p c b", c=2))
            hprev0 = hseq_sb[:, last * NH:(last + 1) * NH]
            nc.gpsimd.tensor_scalar_mul(h9[1][:], hprev0, 1.0 - ALPHA)
            st6b = wk.tile([BL, 6], fp, tag="st6")
            st2b = wk.tile([BL, 2], fp, tag="st2")
            sveb = wk.tile([BL, 1], fp, tag="sve")
            nc.vector.bn_stats(st6b[:], hprev0)
            nc.vector.bn_aggr(st2b[:], st6b[:])
            nc.scalar.activation(sveb[:], st2b[:, 1:2], AF.Sqrt, bias=epsc[:])
            nc.vector.reciprocal(rstd[1][:], sveb[:])

            ET = mybir.EngineType
            with tc.For_i(0, T, TC,
                          hint_engines=(ET.PE, ET.Activation, ET.DVE,
                                        ET.Pool, ET.SP),
                          staggered_reset=True) as iv:
                # x chunk in
                xbuf = xp.tile([BL, TC * NX], fp, tag="xbuf")
                nc.sync.dma_start(xbuf[:], x[:, bass.ds(iv, TC), :])

                # x transpose prepass: [32,64] per t -> xT [64, t*32+b]
                for g in range(TC // 2):
                    xt_ps = pmm.tile([NX, 2 * BL], fp, tag="xt", bufs=1)
                    for u in range(2):
                        t = 2 * g + u
                        nc.tensor.transpose(
                            xt_ps[:, u * BL:(u + 1) * BL],
                            xbuf[:, t * NX:(t + 1) * NX], idt[:, :])
                    nc.scalar.copy(
                        xT_sb[:, 2 * g * BL:(2 * g + 2) * BL], xt_ps[:])

                # X2 = x_t @ Wi2h.T, pairs of steps share one PSUM bank,
                # then one ACT copy to SBUF (STT reads only 1 PSUM input,
                # and partition bases of its SB inputs must match)
                x2_tiles = []
                for q in range(TC // 2):
                    x2_ps = pmm.tile([BL, 2 * NH], fp, tag="x2")
                    for u in range(2):
                        nc.tensor.matmul(
                            x2_ps[:, u * NH:(u + 1) * NH],
                            xT_sb[:, (2 * q + u) * BL:(2 * q + u + 1) * BL],
                            wi[:], start=True, stop=True)
                    x2c = wk.tile([BL, 2 * NH], fp, tag="x2sb", bufs=4,
                                  name=f"x2c{q}")
                    nc.scalar.copy(x2c[:], x2_ps[:])
                    x2_tiles.append(x2c)

                y_ps = None
                for t in range(TC):
                    pv = (t + TC - 1) % TC
                    cur = t % 2
                    prv = (t + 1) % 2
                    h_prev = hseq_sb[:, pv * NH:(pv + 1) * NH]
                    # hseqT col layout: (t//4)*256 + c*128 + (t%4)*32 + b
                    hsv = hseqT_sb.rearrange("p (g c x) -> p g c x",
                                             c=2, x=4 * BL)
                    gp, tp = pv // 4, pv % 4

                    # G = h_prev @ W'.T  (2 K-chunks accumulate)
                    g_ps = pmm.tile([BL, NH], fp, tag="g")
                    nc.tensor.matmul(
                        g_ps[:], hsv[:, gp, 0, tp * BL:(tp + 1) * BL],
                        wp0[:], start=True, stop=False)
                    nc.tensor.matmul(
                        g_ps[:], hsv[:, gp, 1, tp * BL:(tp + 1) * BL],
                        wp1[:], start=False, stop=True)
                    # z = G*rstd + X2
                    z = wk.tile([BL, NH], fp, tag="z")
                    x2c = x2_tiles[t // 2]
                    nc.vector.scalar_tensor_tensor(
                        z[:], g_ps[:], rstd[prv][:],
                        x2c[:, (t % 2) * NH:(t % 2 + 1) * NH],
                        op0=AL.mult, op1=AL.add)
                    # fh = sigmoid(z)
                    fh = wk.tile([BL, NH], fp, tag="fh")
                    nc.scalar.activation(fh[:], z[:], AF.Sigmoid)
                    # h_new = 0.1*fh + 0.9*h_prev  -> hseq slice t
                    h_new = hseq_sb[:, t * NH:(t + 1) * NH]
                    nc.vector.scalar_tensor_tensor(
                        h_new, fh[:], ALPHA, h9[prv][:],
                        op0=AL.mult, op1=AL.add)
                    # 0.9*h_new for next step (gpsimd)
                    nc.gpsimd.tensor_scalar_mul(h9[cur][:], h_new, 1.0 - ALPHA)
                    # rstd of h_new
                    st6 = wk.tile([BL, 6], fp, tag="st6")
                    nc.vector.bn_stats(st6[:], h_new)
                    st2 = wk.tile([BL, 2], fp, tag="st2")
                    nc.vector.bn_aggr(st2[:], st6[:])
                    sve = wk.tile([BL, 1], fp, tag="sve")
                    nc.scalar.activation(sve[:], st2[:, 1:2], AF.Sqrt,
                                         bias=epsc[:])
                    nc.vector.reciprocal(rstd[cur][:], sve[:])
                    # transpose h_new into T-state
                    hT_ps = pt.tile([128, 2 * BL], fp, tag="hT")
                    nc.tensor.transpose(
                        hT_ps[:, 0:BL], hseq_sb[:, t * NH:t * NH + 128],
                        idt[:, :])
                    nc.tensor.transpose(
                        hT_ps[:, BL:2 * BL],
                        hseq_sb[:, t * NH + 128:(t + 1) * NH], idt[:, :])
                    nc.scalar.copy(
                        hsv[:, t // 4, :, (t % 4) * BL:(t % 4 + 1) * BL],
                        hT_ps.rearrange("p (c b) -> p c b", c=2))
                    # y = h_new @ Wh2y.T, M-packed 4 steps: rows (tt,b)=128,
                    # de-interleave happens in the output DMA access pattern
                    if t % 4 == 3:
                        q = t // 4
                        y_ps = py.tile([128, NY], fp, tag="y")
                        for c in range(2):
                            nc.tensor.matmul(
                                y_ps[:], hsv[:, t // 4, c, :],
                                [wy0, wy1][c][:],
                                start=(c == 0), stop=(c == 1))
                        nc.scalar.copy(y_sb[:, q * NY:(q + 1) * NY], y_ps[:])
                    # hseq/y halves out
                    if t == TC // 2 - 1:
                        nc.sync.dma_start(
                            hseq_d[:, bass.ds(iv, TC // 2), :],
                            hseq_sb[:, 0:(TC // 2) * NH])
                    elif t == TC - 1:
                        nc.sync.dma_start(
                            hseq_d[:, bass.ds(iv + TC // 2, TC // 2), :],
                            hseq_sb[:, (TC // 2) * NH:])
                        yv = y_d[:, bass.ds(iv, TC), :].rearrange(
                            "b (q tt) j -> tt b q j", tt=4)
                        for tt in range(4):
                            nc.sync.dma_start(
                                yv[tt], y_sb[tt * BL:(tt + 1) * BL, :])
    nc.finalize()
    return nc


_NC_CACHE = None


def kernel(x, h, Wi2h, Wh2h, Wh2y):
    global _NC_CACHE, LAST_RESULT
    from concourse import bass_utils

    x = np.ascontiguousarray(np.asarray(x, dtype=np.float32))
    h = np.ascontiguousarray(np.asarray(h, dtype=np.float32))
    Wi2h = np.asarray(Wi2h, dtype=np.float32)
    Wh2h = np.asarray(Wh2h, dtype=np.float32)
    Wh2y = np.asarray(Wh2y, dtype=np.float32)

    rs = Wh2h.sum(axis=1, keepdims=True)
    wpT = np.ascontiguousarray((Wh2h - rs / NH).T)
    wiT = np.ascontiguousarray(Wi2h.T)
    wyT = np.ascontiguousarray(Wh2y.T)
    ident = np.eye(BL, dtype=np.float32)

    if _NC_CACHE is None:
        _NC_CACHE = build_bass()
    nc = _NC_CACHE

    in_maps = []
    for c in range(NCORES):
        in_maps.append({
            "x": np.ascontiguousarray(x[c * BL:(c + 1) * BL]),
            "h0": np.ascontiguousarray(h[c * BL:(c + 1) * BL]),
            "wpT": wpT, "wiT": wiT, "wyT": wyT, "ident": ident,
        })

    res = bass_utils.run_bass_kernel_spmd(
        nc, in_maps, core_ids=list(range(NCORES)), trace=TRACE)
    LAST_RESULT = {"exec_time_ns": res.exec_time_ns,
                   "trace": res.instructions_and_trace}
    y = np.concatenate([r["y"] for r in res.results], axis=0)
    hseq = np.concatenate([r["hseq"] for r in res.results], axis=0)
    return y, hseq
